# revision 21
# baseline (speedup 1.0000x reference)
"""Trainium2 Bass kernel for nn_MoDEChameleonMLP (MoDE Chameleon MLP).

Math (per token n):
  gate = x@Wg.T + delta_g(x); up = x@Wu.T + delta_u(x)
  inter = silu(gate)*up
  out  = inter@Wd.T + delta_d(inter)
where delta(v) = mask ? 2*(v@vA.T)@vB.T : 2*sum_e softmax(v@router.T)_e (v@A_e.T)@B_e.T

Implementation: token(B*S)-sharding across 8 cores (512 tokens/core, no
collectives). Each core:
  aux:    t = x@Acat.T (rank-40 LoRA bases + router logits for gate/up),
          h-chunked over the streaming x DMA, all four token-tiles packed
          into a single PSUM bank (one start zeroes the bank; disjoint
          column ranges accumulate independently). Routing (softmax +
          mask combine) on DVE, transposed to [40,T] via PE+identity.
  phase1: gate/up = W-stationary matmuls producing [I-part, token] tiles,
          plus one extra K=128 matmul with the (pre-scaled) LoRA B matrix
          and y -> the full delta. s=0's weight DMAs and gate psum tags
          are pre-reserved before the aux scope so neither SBUF nor PSUM
          address reuse serializes phase1 behind the routing chains.
  phase2: down projection in 9 column-chunks; the first chunk's weight
          tile carries 44 extra columns holding the down-routing A
          matrices, so the whole down aux projection rides the existing
          matmuls (+1% rows) instead of a separate 344-matmul pass.
          Down-routing reads those PSUM columns directly; its transposes
          borrow the second generation of the output psum rings.
All matmuls bf16 with fp32 PSUM accumulation. Weights are host-side
transposed/pre-tiled so every device DMA is wide contiguous lines.
"""
import os
import sys

for p in ("/root/.axon_site/_ro/trn_rl_repo", "/opt/trn_rl_repo"):
    if os.path.isdir(p) and p not in sys.path:
        sys.path.append(p)

import numpy as np
import ml_dtypes

import concourse.bass as bass  # noqa: E402
import concourse.tile as tile  # noqa: E402
from concourse import bacc, mybir  # noqa: E402
from concourse.bass_utils import run_bass_kernel_spmd  # noqa: E402
from concourse.masks import make_identity  # noqa: E402

BF16 = ml_dtypes.bfloat16
BF = mybir.dt.bfloat16
F32 = mybir.dt.float32

NCORES = 8
T = 512          # tokens per core
TT = T // 128
SW = 256         # i-super width (2 x 128 psum tiles), divides 11008
E, R = 4, 8
SCALE = 2.0

# phase2 column chunking: chunk 0 carries 352 h-cols + 44 down-aux cols,
# then seven 512-wide chunks and a narrow 160-wide tail chunk (so the
# final psum drain after the last matmul is short). 352+7*512+160 = 4096.
CH_H = ([(0, 352)] + [(352 + 512 * k, 352 + 512 * (k + 1)) for k in range(7)]
        + [(3936, 4096)])
CH_F = [396] + [512] * 7 + [160]  # psum free width per chunk
N_CH = len(CH_H)

_nc_cache = {}


def build_kernel(H, I):
    HB, IB = H // 128, I // 128
    NS = I // SW
    NI2 = SW // 128
    XC = 4                        # xt dma chunks
    HCB = HB // XC                # h-blocks per xt chunk
    QW = 8                        # h-blocks per weight dma descriptor
    NQ = HB // QW
    wd_off = [86 * sum(CH_F[:c]) for c in range(N_CH)]
    WD_TOT = 86 * sum(CH_F)

    nc = bacc.Bacc(None, target_bir_lowering=False)
    xt_d = nc.declare_dram_parameter("xt", [128, HB, T], BF, isOutput=False)
    mask_d = nc.declare_dram_parameter("maskf", [128, 2 * TT], F32, isOutput=False)
    acall_d = nc.declare_dram_parameter("acatall", [128, HB, 88], BF, isOutput=False)
    wg_d = nc.declare_dram_parameter("wg", [NS, 128, HB, SW], BF, isOutput=False)
    wu_d = nc.declare_dram_parameter("wu", [NS, 128, HB, SW], BF, isOutput=False)
    bg_d = nc.declare_dram_parameter("bg", [NS, 128, SW], BF, isOutput=False)
    bu_d = nc.declare_dram_parameter("bu", [NS, 128, SW], BF, isOutput=False)
    wdcat_d = nc.declare_dram_parameter("wdcat", [128, WD_TOT], BF, isOutput=False)
    bdcat_d = nc.declare_dram_parameter("bdcat", [128, H], BF, isOutput=False)
    out_d = nc.declare_dram_parameter("out", [T, H], F32, isOutput=True)

    with tile.TileContext(nc) as tc:
        # wstr/wstr2/bstr2 are opened up-front so their SBUF addresses are
        # disjoint from every scoped pool: their DMAs then have no
        # address-release deps and prefetch freely across phase boundaries.
        with tc.tile_pool(name="const", bufs=1) as constp, \
             tc.tile_pool(name="wstr", bufs=10) as wstr, \
             tc.tile_pool(name="wstr2", bufs=10) as wstr2, \
             tc.tile_pool(name="bstr2", bufs=2) as bstr2:
            # ---- input DMAs, issue order matters: acall + xt chunks first
            # so the aux pass starts ASAP; s=0 weights follow immediately.
            acallc, xtc, wt_g0 = [], [], []

            def wg0_tile(q):
                wq = wstr.tile([128, QW, SW], BF, tag="wt", name=f"wg0_{q}")
                nc.sync.dma_start(wq[:], wg_d[0, :, q * QW:(q + 1) * QW, :])
                wt_g0.append(wq)

            for c in range(XC):
                ac = constp.tile([128, HCB, 88], BF, tag=f"acc{c}",
                                 name=f"acallc{c}")
                nc.sync.dma_start(ac[:], acall_d[:, c * HCB:(c + 1) * HCB, :])
                acallc.append(ac)
                xs = constp.tile([128, HCB, T], BF, tag=f"xtc{c}", name=f"xtc{c}")
                nc.sync.dma_start(xs[:], xt_d[:, c * HCB:(c + 1) * HCB, :])
                xtc.append(xs)
                wg0_tile(c)
            mask_sb = constp.tile([128, 2 * TT], F32)
            nc.sync.dma_start(mask_sb[:], mask_d[:])

            def xth(h):
                return xtc[h // HCB][:, h % HCB, :]

            def acall_h(h):
                return acallc[h // HCB][:, h % HCB, :]

            ident = constp.tile([128, 128], BF)
            make_identity(nc, ident)
            ygT = constp.tile([128, T], BF)
            warm_pending = True
            yuT = constp.tile([128, T], BF)
            ydT = constp.tile([128, T], BF)
            for y in (ygT, yuT, ydT):
                nc.vector.memset(y[:], 0.0)
            inter_sb = constp.tile([128, IB, T], BF)

            def emit_route(tpp, tptag, tpbufs, auxtmp, ps, lo, vo, eo, t, yT):
                """softmax(ps[:,lo:lo+4]) routing + mask combine -> y, then
                transpose y[128,40] into yT[0:40, t*128:(t+1)*128] via PE."""
                # router logits are tiny (|l| < ~4: x~0.5-std vs 0.02-std
                # router rows), so plain exp cannot overflow — skip the
                # max-subtraction and fold rec*(1-mask) into one dual-scalar
                # op: two fewer serial ops on the routing critical path.
                ee = auxtmp.tile([128, 4], F32, tag="ee", name=f"ee{t}")
                se = auxtmp.tile([128, 1], F32, tag="se", name=f"se{t}")
                nc.scalar.activation(ee, ps[:, lo:lo + 4],
                                     mybir.ActivationFunctionType.Exp,
                                     accum_out=se)
                rec = auxtmp.tile([128, 1], F32, tag="rec", name=f"rc{t}")
                nc.vector.reciprocal(rec, se)
                we = auxtmp.tile([128, 4], F32, tag="we", name=f"we{t}")
                nc.vector.tensor_scalar(we, ee, rec,
                                        mask_sb[:, TT + t:TT + t + 1],
                                        mybir.AluOpType.mult,
                                        mybir.AluOpType.mult)
                yt = auxtmp.tile([128, 40], BF, tag="yt", name=f"yt{t}")
                nc.vector.tensor_scalar(yt[:, 0:8], ps[:, vo:vo + 8],
                                        mask_sb[:, t:t + 1], None,
                                        mybir.AluOpType.mult)
                for j in range(E):
                    nc.vector.tensor_scalar(yt[:, 8 + 8 * j:16 + 8 * j],
                                            ps[:, eo + 8 * j:eo + 8 * (j + 1)],
                                            we[:, j:j + 1], None,
                                            mybir.AluOpType.mult)
                tp = tpp.tile([128, 128], BF, tag=tptag, name=f"tp{t}",
                              bufs=tpbufs)
                nc.tensor.transpose(tp[:40, :], yt[:], ident)
                nc.vector.tensor_copy(yT[0:40, t * 128:(t + 1) * 128], tp[:40, :])

            with tc.tile_pool(name="bstr", bufs=3) as bstr, \
                 tc.tile_pool(name="etmp", bufs=4) as etmp, \
                 tc.tile_pool(name="ost", bufs=2) as ost, \
                 tc.tile_pool(name="auxtmp", bufs=3) as auxtmp, \
                 tc.tile_pool(name="mps", bufs=2, space="PSUM") as mps:

                def proj_weight_dmas(s, proj, w_dram, b_dram):
                    wt = []
                    for q in range(NQ):
                        wq = wstr.tile([128, QW, SW], BF, tag="wt",
                                       name=f"w{proj}{s}_{q}")
                        nc.sync.dma_start(wq[:], w_dram[s, :, q * QW:(q + 1) * QW, :])
                        wt.append(wq)
                    bt = bstr.tile([128, SW], BF, tag="bt", name=f"b{proj}{s}")
                    nc.sync.dma_start(bt[:], b_dram[s])
                    return wt, bt

                def proj_mains(pss, wt):
                    for h in range(HB):
                        for i2 in range(NI2):
                            nc.tensor.matmul(pss[i2],
                                             wt[h // QW][:, h % QW,
                                                         i2 * 128:(i2 + 1) * 128],
                                             xth(h),
                                             start=(h == 0), stop=False)

                def proj_delta(pss, bt, yT):
                    for i2 in range(NI2):
                        nc.tensor.matmul(pss[i2], bt[:, i2 * 128:(i2 + 1) * 128],
                                         yT[:], start=False, stop=True)

                # wg0 was issued interleaved with the xt chunks above; the
                # rest of s=0's weights stream right behind it.
                bt_g0 = bstr.tile([128, SW], BF, tag="bt", name="bg0")
                nc.sync.dma_start(bt_g0[:], bg_d[0])
                wt_u0, bt_u0 = proj_weight_dmas(0, "u", wu_d, bu_d)

                # ---- aux scope: the packed aux bank + route-transpose
                # staging (3 psum banks). s0's gate psums claim the pg tags
                # of the main ring first; the pu tags are claimed after this
                # scope closes and reuse its banks (Tile inserts the
                # address anti-deps, which are long satisfied by then).
                psg0 = [mps.tile([128, 512], F32, tag=f"pg{i2}",
                                 name=f"pg0_{i2}") for i2 in range(NI2)]

                def emit_silu(s, psg):
                    sts = []
                    for i2 in range(NI2):
                        st = etmp.tile([128, T], F32, tag="silu",
                                       name=f"si{s}_{i2}")
                        nc.scalar.activation(st[:], psg[i2][:, :T],
                                             mybir.ActivationFunctionType.Silu)
                        sts.append(st)
                    return sts

                def emit_mult(s, sts, psu):
                    for i2 in range(NI2):
                        i = s * NI2 + i2
                        nc.vector.tensor_tensor(inter_sb[:, i, :], sts[i2][:],
                                                psu[i2][:, :T],
                                                mybir.AluOpType.mult)

                # aux bank = pu0's first generation; route-transpose staging
                # rides pu1's ring. Everything lives in the one 8-bank pool,
                # so there are no psum pool boundaries anywhere.
                warm = mps.tile([128, 128], BF, tag="pg0", name="warm")
                for w in range(24):
                    nc.tensor.transpose(warm[:], ident, ident)
                auxpk = mps.tile([128, 512], F32, tag="pu0", name="auxpk")
                for hc in range(XC):
                    for t in range(TT):
                        for h in range(hc * HCB, (hc + 1) * HCB):
                            nc.tensor.matmul(
                                auxpk[:, t * 128:t * 128 + 88],
                                xth(h)[:, t * 128:(t + 1) * 128],
                                acall_h(h),
                                start=(t == 0 and h == 0),
                                stop=(t == TT - 1 and h == HB - 1))
                    # s0 gate mains ride along each xt chunk as it lands
                    for h in range(hc * HCB, (hc + 1) * HCB):
                        for i2 in range(NI2):
                            nc.tensor.matmul(psg0[i2],
                                             wt_g0[h // QW][:, h % QW,
                                                            i2 * 128:(i2 + 1) * 128],
                                             xth(h),
                                             start=(h == 0), stop=False)
                for t in range(TT):
                    emit_route(mps, "pu1", 2, auxtmp,
                               auxpk[:, t * 128:(t + 1) * 128],
                               80, 0, 8, t, ygT)
                proj_delta(psg0, bt_g0, ygT)
                st0 = emit_silu(0, psg0)
                for t in range(TT):
                    emit_route(mps, "pu1", 2, auxtmp,
                               auxpk[:, t * 128:(t + 1) * 128],
                               84, 40, 48, t, yuT)
                psu0 = [mps.tile([128, 512], F32, tag=f"pu{i2}",
                                 name=f"pu0_{i2}") for i2 in range(NI2)]
                proj_mains(psu0, wt_u0)
                proj_delta(psu0, bt_u0, yuT)
                emit_mult(0, st0, psu0)

                if True:
                    pre_bd0, pre_wd0 = None, []
                    for s in range(1, NS):
                        wt_g, bt_g = proj_weight_dmas(s, "g", wg_d, bg_d)
                        psg = [mps.tile([128, 512], F32, tag=f"pg{i2}",
                                        name=f"pg{s}_{i2}") for i2 in range(NI2)]
                        proj_mains(psg, wt_g)
                        proj_delta(psg, bt_g, ygT)
                        sts = emit_silu(s, psg)
                        wt_u, bt_u = proj_weight_dmas(s, "u", wu_d, bu_d)
                        psu = [mps.tile([128, 512], F32, tag=f"pu{i2}",
                                        name=f"pu{s}_{i2}") for i2 in range(NI2)]
                        proj_mains(psu, wt_u)
                        proj_delta(psu, bt_u, yuT)
                        emit_mult(s, sts, psu)
                        if s == 20:
                            # prefetch the first phase-2 tiles so chunk 0
                            # starts without waiting on the Sync queue.
                            a0, b0 = CH_H[0]
                            pre_bd0 = bstr2.tile([128, 512], BF, tag="bd2",
                                                 name="bd0")
                            nc.sync.dma_start(pre_bd0[:, :b0 - a0],
                                              bdcat_d[:, a0:b0])
                            wc0 = CH_F[0]
                            for ip in range(4):
                                wdt = wstr2.tile([128, 2 * 512], BF, tag="wd2",
                                                 name=f"wd0_{ip}")
                                nc.sync.dma_start(
                                    wdt[:, :2 * wc0],
                                    wdcat_d[:, wd_off[0] + ip * 2 * wc0:
                                            wd_off[0] + (ip + 1) * 2 * wc0])
                                pre_wd0.append(wdt)

                # ---- phase 2: down projection in N_CH column chunks,
                # sharing the SAME psum tag rings as phase 1 (no pool
                # boundary: chunk 0's psum slots were released back at
                # s=41, so phase 2 starts without draining phase 1).
                PT = ["pg0", "pg1", "pu0", "pu1"]
                out_v = out_d.rearrange("(t p) h -> p t h", p=128)

                def finish(c, pso, bdt):
                    a, b = CH_H[c]
                    wh = b - a
                    if c == N_CH - 1:
                        # combined store: one osb tile, copies alternating
                        # DVE/ACT, a single output descriptor.
                        osb = ost.tile([128, TT, 160], F32, tag="osl",
                                       name=f"osl{c}", bufs=1)
                        for t in range(TT):
                            nc.tensor.matmul(pso[t][:, :wh],
                                             ydT[:, t * 128:(t + 1) * 128],
                                             bdt[:, :wh], start=False, stop=True)
                            if t % 2 == 0:
                                nc.vector.tensor_copy(osb[:, t, :wh],
                                                      pso[t][:, :wh])
                            else:
                                nc.scalar.activation(
                                    osb[:, t, :wh], pso[t][:, :wh],
                                    mybir.ActivationFunctionType.Copy)
                        nc.sync.dma_start(out_v[:, :, a:b], osb[:])
                        return
                    for t in range(TT):
                        nc.tensor.matmul(pso[t][:, :wh],
                                         ydT[:, t * 128:(t + 1) * 128],
                                         bdt[:, :wh], start=False, stop=True)
                        osb = ost.tile([128, 512], F32, tag="os", name=f"os{c}_{t}")
                        nc.vector.tensor_copy(osb[:, :wh], pso[t][:, :wh])
                        nc.sync.dma_start(
                            out_d[t * 128:(t + 1) * 128, a:b], osb[:, :wh])

                for c in range(N_CH):
                    a, b = CH_H[c]
                    wh, wc = b - a, CH_F[c]
                    gp = 2 if wc >= 396 else 4   # i-blocks per descriptor
                    if c == 0 and pre_bd0 is not None:
                        bdt = pre_bd0
                    else:
                        bdt = bstr2.tile([128, 512], BF, tag="bd2",
                                         name=f"bd{c}")
                        nc.sync.dma_start(bdt[:, :wh], bdcat_d[:, a:b])
                    pso = [mps.tile([128, 512], F32, tag=PT[t],
                                    name=f"po{c}_{t}") for t in range(TT)]
                    for ip in range((IB + gp - 1) // gp):
                        ni = min(gp, IB - ip * gp)
                        if c == 0 and ip < len(pre_wd0):
                            wdt = pre_wd0[ip]
                        else:
                            wdt = wstr2.tile([128, 2 * 512], BF, tag="wd2",
                                             name=f"wd{c}_{ip}")
                            nc.sync.dma_start(
                                wdt[:, :ni * wc],
                                wdcat_d[:, wd_off[c] + ip * gp * wc:
                                        wd_off[c] + (ip * gp + ni) * wc])
                        for j in range(ni):
                            i = gp * ip + j
                            for t in range(TT):
                                nc.tensor.matmul(
                                    pso[t][:, :wc],
                                    inter_sb[:, i, t * 128:(t + 1) * 128],
                                    wdt[:, j * wc:(j + 1) * wc],
                                    start=(i == 0), stop=False)
                    if c == 0:
                        # down-routing straight from the psum aux columns;
                        # the transposes borrow the next ring generation.
                        for t in range(TT):
                            emit_route(mps, PT[t], 2, auxtmp, pso[t],
                                       352, 356, 364, t, ydT)
                    finish(c, pso, bdt)
    nc.finalize()
    return nc


def get_nc(H, I):
    key = (H, I)
    if key not in _nc_cache:
        _nc_cache[key] = build_kernel(H, I)
    return _nc_cache[key]


def _prep_weights(Wg, Wu, Wd, va_gate_A, va_gate_B, va_up_A, va_up_B,
                  va_down_A, va_down_B, router_gate, tm_gate_A, tm_gate_B,
                  router_up, tm_up_A, tm_up_B, router_down, tm_down_A, tm_down_B):
    I, H = Wg.shape
    HB, IB = H // 128, I // 128
    NS = I // SW

    def tile_w_ih(W):  # [I,H] -> [NS,128,HB,SW]; w[s,p,h,c]=W[s*SW+c, h*128+p]
        return np.ascontiguousarray(
            W.reshape(NS, SW, HB, 128).transpose(0, 3, 2, 1)).astype(BF16)

    def tile_bcat(vB, tB, rows):  # -> [nblk,128,blk]; padded 2*[vB|tB_e].T
        out_dim = vB.shape[0]
        Bcat = np.concatenate([vB] + [tB[e] for e in range(E)], axis=1)  # [out,40]
        Bp = np.zeros((128, out_dim), np.float32)
        Bp[:40, :] = SCALE * Bcat.T
        blk = out_dim // rows
        return np.ascontiguousarray(
            Bp.reshape(128, rows, blk).transpose(1, 0, 2)).astype(BF16)

    A_all = np.concatenate([va_gate_A, tm_gate_A.reshape(E * R, H),
                            va_up_A, tm_up_A.reshape(E * R, H),
                            router_gate, router_up], axis=0)  # [88,H]
    acatall = np.ascontiguousarray(
        A_all.T.reshape(HB, 128, 88).transpose(1, 0, 2)).astype(BF16)
    A_d = np.concatenate([router_down, va_down_A,
                          tm_down_A.reshape(E * R, I)], axis=0)  # [44,I]

    # down weights in column chunks; chunk 0 carries the down-aux columns
    parts = []
    for c, (a, b) in enumerate(CH_H):
        cols = Wd[a:b, :]                                    # [wh, I]
        if c == 0:
            cols = np.concatenate([cols, A_d], axis=0)       # [wh+44, I]
        wc = cols.shape[0]
        t = cols.T.reshape(IB, 128, wc).transpose(1, 0, 2)   # [128,IB,wc]
        parts.append(t.reshape(128, IB * wc))
    wdcat = np.ascontiguousarray(np.concatenate(parts, axis=1)).astype(BF16)

    Bcat_d = np.concatenate([va_down_B] + [tm_down_B[e] for e in range(E)],
                            axis=1)                          # [H,40]
    bdcat = np.zeros((128, H), np.float32)
    bdcat[:40, :] = SCALE * Bcat_d.T
    bdcat = np.ascontiguousarray(bdcat).astype(BF16)

    return {
        "acatall": acatall,
        "wg": tile_w_ih(Wg),
        "wu": tile_w_ih(Wu),
        "bg": tile_bcat(va_gate_B, tm_gate_B, NS),
        "bu": tile_bcat(va_up_B, tm_up_B, NS),
        "wdcat": wdcat,
        "bdcat": bdcat,
    }


def _prep_core_inputs(x, image_mask, weights, n_cores):
    Bb, S, H = x.shape
    HB = H // 128
    xf = np.asarray(x, np.float32).reshape(-1, H)
    m = np.asarray(image_mask).reshape(-1).astype(np.float32)
    in_maps = []
    for c in range(n_cores):
        sh = xf[c * T:(c + 1) * T]                      # [T,H]
        xt = np.ascontiguousarray(
            sh.T.reshape(HB, 128, T).transpose(1, 0, 2)).astype(BF16)
        mc = m[c * T:(c + 1) * T].reshape(TT, 128).T    # [128,TT]
        maskf = np.ascontiguousarray(
            np.concatenate([mc, 1.0 - mc], axis=1)).astype(np.float32)
        in_maps.append({"xt": xt, "maskf": maskf, **weights})
    return in_maps


def run(x, image_mask, weights_raw, trace=False):
    Bb, S, H = x.shape
    I = weights_raw["Wg"].shape[0]
    nc = get_nc(H, I)
    weights = _prep_weights(**weights_raw)
    in_maps = _prep_core_inputs(x, image_mask, weights, NCORES)
    res = run_bass_kernel_spmd(nc, in_maps, list(range(NCORES)), trace=trace)
    out = np.concatenate([r["out"] for r in res.results], axis=0)
    return out.reshape(Bb, S, H).astype(np.float32), res


def kernel(x, image_mask, Wg, Wu, Wd,
           va_gate_A, va_gate_B, va_up_A, va_up_B, va_down_A, va_down_B,
           router_gate, tm_gate_A, tm_gate_B,
           router_up, tm_up_A, tm_up_B,
           router_down, tm_down_A, tm_down_B):
    weights_raw = dict(
        Wg=np.asarray(Wg, np.float32), Wu=np.asarray(Wu, np.float32),
        Wd=np.asarray(Wd, np.float32),
        va_gate_A=np.asarray(va_gate_A), va_gate_B=np.asarray(va_gate_B),
        va_up_A=np.asarray(va_up_A), va_up_B=np.asarray(va_up_B),
        va_down_A=np.asarray(va_down_A), va_down_B=np.asarray(va_down_B),
        router_gate=np.asarray(router_gate), tm_gate_A=np.asarray(tm_gate_A),
        tm_gate_B=np.asarray(tm_gate_B),
        router_up=np.asarray(router_up), tm_up_A=np.asarray(tm_up_A),
        tm_up_B=np.asarray(tm_up_B),
        router_down=np.asarray(router_down), tm_down_A=np.asarray(tm_down_A),
        tm_down_B=np.asarray(tm_down_B),
    )
    out, _ = run(np.asarray(x), np.asarray(image_mask), weights_raw, trace=False)
    return out


# revision 23
# speedup vs baseline: 1.1119x; 1.1119x over previous
"""Trainium2 Bass kernel for nn_MoDEChameleonMLP (MoDE Chameleon MLP).

Math (per token n):
  gate = x@Wg.T + delta_g(x); up = x@Wu.T + delta_u(x)
  inter = silu(gate)*up
  out  = inter@Wd.T + delta_d(inter)
where delta(v) = mask ? 2*(v@vA.T)@vB.T : 2*sum_e softmax(v@router.T)_e (v@A_e.T)@B_e.T

Implementation: token(B*S)-sharding across 8 cores (512 tokens/core, no
collectives). Each core:
  aux:    t = x@Acat.T (rank-40 LoRA bases + router logits for gate/up),
          h-chunked over the streaming x DMA, all four token-tiles packed
          into a single PSUM bank (one start zeroes the bank; disjoint
          column ranges accumulate independently). Routing (softmax +
          mask combine) on DVE, transposed to [40,T] via PE+identity.
  phase1: gate/up = W-stationary matmuls producing [I-part, token] tiles,
          plus one extra K=128 matmul with the (pre-scaled) LoRA B matrix
          and y -> the full delta. s=0's weight DMAs and gate psum tags
          are pre-reserved before the aux scope so neither SBUF nor PSUM
          address reuse serializes phase1 behind the routing chains.
  phase2: down projection in 9 column-chunks; the first chunk's weight
          tile carries 44 extra columns holding the down-routing A
          matrices, so the whole down aux projection rides the existing
          matmuls (+1% rows) instead of a separate 344-matmul pass.
          Down-routing reads those PSUM columns directly; its transposes
          borrow the second generation of the output psum rings.
All matmuls bf16 with fp32 PSUM accumulation. Weights are host-side
transposed/pre-tiled so every device DMA is wide contiguous lines.
"""
import os
import sys

for p in ("/root/.axon_site/_ro/trn_rl_repo", "/opt/trn_rl_repo"):
    if os.path.isdir(p) and p not in sys.path:
        sys.path.append(p)

import numpy as np
import ml_dtypes

import concourse.bass as bass  # noqa: E402
import concourse.tile as tile  # noqa: E402
from concourse import bacc, mybir  # noqa: E402
from concourse.bass_utils import run_bass_kernel_spmd  # noqa: E402
from concourse.masks import make_identity  # noqa: E402

BF16 = ml_dtypes.bfloat16
BF = mybir.dt.bfloat16
F32 = mybir.dt.float32

NCORES = 8
T = 512          # tokens per core
TT = T // 128
SW = 256         # i-super width (2 x 128 psum tiles), divides 11008
E, R = 4, 8
SCALE = 2.0

# phase2 column chunking: chunk 0 carries 352 h-cols + 44 down-aux cols,
# then seven 512-wide chunks and a narrow 160-wide tail chunk (so the
# final psum drain after the last matmul is short). 352+7*512+160 = 4096.
CH_H = ([(0, 352)] + [(352 + 512 * k, 352 + 512 * (k + 1)) for k in range(7)]
        + [(3936, 4096)])
CH_F = [396] + [512] * 7 + [160]  # psum free width per chunk
N_CH = len(CH_H)

_nc_cache = {}


def build_kernel(H, I):
    HB, IB = H // 128, I // 128
    NS = I // SW
    NI2 = SW // 128
    XC = 4                        # xt dma chunks
    HCB = HB // XC                # h-blocks per xt chunk
    QW = 8                        # h-blocks per weight dma descriptor
    NQ = HB // QW
    wd_off = [86 * sum(CH_F[:c]) for c in range(N_CH)]
    WD_TOT = 86 * sum(CH_F)

    nc = bacc.Bacc(None, target_bir_lowering=False)
    xt_d = nc.declare_dram_parameter("xt", [128, HB, T], BF, isOutput=False)
    mask_d = nc.declare_dram_parameter("maskf", [128, 2 * TT], F32, isOutput=False)
    acall_d = nc.declare_dram_parameter("acatall", [128, HB, 88], BF, isOutput=False)
    wg_d = nc.declare_dram_parameter("wg", [NS, 128, HB, SW], BF, isOutput=False)
    wu_d = nc.declare_dram_parameter("wu", [NS, 128, HB, SW], BF, isOutput=False)
    bg_d = nc.declare_dram_parameter("bg", [NS, 128, SW], BF, isOutput=False)
    bu_d = nc.declare_dram_parameter("bu", [NS, 128, SW], BF, isOutput=False)
    wdcat_d = nc.declare_dram_parameter("wdcat", [128, WD_TOT], BF, isOutput=False)
    bdcat_d = nc.declare_dram_parameter("bdcat", [128, H], BF, isOutput=False)
    out_d = nc.declare_dram_parameter("out", [T, H], F32, isOutput=True)

    with tile.TileContext(nc) as tc:
        # wstr/wstr2/bstr2 are opened up-front so their SBUF addresses are
        # disjoint from every scoped pool: their DMAs then have no
        # address-release deps and prefetch freely across phase boundaries.
        with tc.tile_pool(name="const", bufs=1) as constp, \
             tc.tile_pool(name="wstr", bufs=10) as wstr, \
             tc.tile_pool(name="wstr2", bufs=10) as wstr2, \
             tc.tile_pool(name="bstr2", bufs=2) as bstr2:
            # ---- input DMAs, issue order matters: acall + xt chunks first
            # so the aux pass starts ASAP; s=0 weights follow immediately.
            acallc, xtc, wt_g0 = [], [], []

            def wg0_tile(q):
                wq = wstr.tile([128, QW, SW], BF, tag="wt", name=f"wg0_{q}")
                nc.sync.dma_start(wq[:], wg_d[0, :, q * QW:(q + 1) * QW, :])
                wt_g0.append(wq)

            for c in range(XC):
                ac = constp.tile([128, HCB, 88], BF, tag=f"acc{c}",
                                 name=f"acallc{c}")
                nc.sync.dma_start(ac[:], acall_d[:, c * HCB:(c + 1) * HCB, :])
                acallc.append(ac)
                xs = constp.tile([128, HCB, T], BF, tag=f"xtc{c}", name=f"xtc{c}")
                nc.sync.dma_start(xs[:], xt_d[:, c * HCB:(c + 1) * HCB, :])
                xtc.append(xs)
                wg0_tile(c)
            mask_sb = constp.tile([128, 2 * TT], F32)
            nc.sync.dma_start(mask_sb[:], mask_d[:])

            def xth(h):
                return xtc[h // HCB][:, h % HCB, :]

            def acall_h(h):
                return acallc[h // HCB][:, h % HCB, :]

            ident = constp.tile([128, 128], BF)
            make_identity(nc, ident)
            ygT = constp.tile([128, T], BF)
            warm_pending = True
            yuT = constp.tile([128, T], BF)
            ydT = constp.tile([128, T], BF)
            for y in (ygT, yuT, ydT):
                nc.vector.memset(y[:], 0.0)
            inter_sb = constp.tile([128, IB, T], BF)

            def emit_route(tpp, tptag, tpbufs, auxtmp, ps, lo, vo, eo, t, yT):
                """softmax(ps[:,lo:lo+4]) routing + mask combine -> y, then
                transpose y[128,40] into yT[0:40, t*128:(t+1)*128] via PE."""
                rmaxn = auxtmp.tile([128, 1], F32, tag="rmaxn", name=f"rx{t}")
                nc.vector.tensor_reduce(rmaxn, ps[:, lo:lo + 4],
                                        axis=mybir.AxisListType.X,
                                        op=mybir.AluOpType.max, negate=True)
                ee = auxtmp.tile([128, 4], F32, tag="ee", name=f"ee{t}")
                se = auxtmp.tile([128, 1], F32, tag="se", name=f"se{t}")
                nc.scalar.activation(ee, ps[:, lo:lo + 4],
                                     mybir.ActivationFunctionType.Exp,
                                     bias=rmaxn, accum_out=se)
                rec = auxtmp.tile([128, 1], F32, tag="rec", name=f"rc{t}")
                nc.vector.reciprocal(rec, se)
                r1m = auxtmp.tile([128, 1], F32, tag="r1m", name=f"rm{t}")
                nc.vector.tensor_tensor(r1m, rec, mask_sb[:, TT + t:TT + t + 1],
                                        mybir.AluOpType.mult)
                we = auxtmp.tile([128, 4], F32, tag="we", name=f"we{t}")
                nc.vector.tensor_scalar(we, ee, r1m, None, mybir.AluOpType.mult)
                yt = auxtmp.tile([128, 40], BF, tag="yt", name=f"yt{t}")
                nc.vector.tensor_scalar(yt[:, 0:8], ps[:, vo:vo + 8],
                                        mask_sb[:, t:t + 1], None,
                                        mybir.AluOpType.mult)
                for j in range(E):
                    nc.vector.tensor_scalar(yt[:, 8 + 8 * j:16 + 8 * j],
                                            ps[:, eo + 8 * j:eo + 8 * (j + 1)],
                                            we[:, j:j + 1], None,
                                            mybir.AluOpType.mult)
                tp = tpp.tile([128, 128], BF, tag=tptag, name=f"tp{t}",
                              bufs=tpbufs)
                nc.tensor.transpose(tp[:40, :], yt[:], ident)
                nc.vector.tensor_copy(yT[0:40, t * 128:(t + 1) * 128], tp[:40, :])

            with tc.tile_pool(name="bstr", bufs=3) as bstr, \
                 tc.tile_pool(name="etmp", bufs=4) as etmp, \
                 tc.tile_pool(name="ost", bufs=2) as ost, \
                 tc.tile_pool(name="auxtmp", bufs=4) as auxtmp, \
                 tc.tile_pool(name="mps", bufs=2, space="PSUM") as mps:

                def proj_weight_dmas(s, proj, w_dram, b_dram):
                    wt = []
                    for q in range(NQ):
                        wq = wstr.tile([128, QW, SW], BF, tag="wt",
                                       name=f"w{proj}{s}_{q}")
                        nc.sync.dma_start(wq[:], w_dram[s, :, q * QW:(q + 1) * QW, :])
                        wt.append(wq)
                    bt = bstr.tile([128, SW], BF, tag="bt", name=f"b{proj}{s}")
                    nc.sync.dma_start(bt[:], b_dram[s])
                    return wt, bt

                def proj_mains(pss, wt):
                    for h in range(HB):
                        for i2 in range(NI2):
                            nc.tensor.matmul(pss[i2],
                                             wt[h // QW][:, h % QW,
                                                         i2 * 128:(i2 + 1) * 128],
                                             xth(h),
                                             start=(h == 0), stop=False)

                def proj_delta(pss, bt, yT):
                    for i2 in range(NI2):
                        nc.tensor.matmul(pss[i2], bt[:, i2 * 128:(i2 + 1) * 128],
                                         yT[:], start=False, stop=True)

                # wg0 was issued interleaved with the xt chunks above; the
                # rest of s=0's weights stream right behind it.
                bt_g0 = bstr.tile([128, SW], BF, tag="bt", name="bg0")
                nc.sync.dma_start(bt_g0[:], bg_d[0])
                wt_u0, bt_u0 = proj_weight_dmas(0, "u", wu_d, bu_d)

                # ---- aux scope: the packed aux bank + route-transpose
                # staging (3 psum banks). s0's gate psums claim the pg tags
                # of the main ring first; the pu tags are claimed after this
                # scope closes and reuse its banks (Tile inserts the
                # address anti-deps, which are long satisfied by then).
                psg0 = [mps.tile([128, 512], F32, tag=f"pg{i2}",
                                 name=f"pg0_{i2}") for i2 in range(NI2)]

                def emit_silu(s, psg):
                    sts = []
                    for i2 in range(NI2):
                        st = etmp.tile([128, T], F32, tag="silu",
                                       name=f"si{s}_{i2}")
                        nc.scalar.activation(st[:], psg[i2][:, :T],
                                             mybir.ActivationFunctionType.Silu)
                        sts.append(st)
                    return sts

                def emit_mult(s, sts, psu):
                    for i2 in range(NI2):
                        i = s * NI2 + i2
                        nc.vector.tensor_tensor(inter_sb[:, i, :], sts[i2][:],
                                                psu[i2][:, :T],
                                                mybir.AluOpType.mult)

                # aux bank = pu0's first generation; route-transpose staging
                # rides pu1's ring. Everything lives in the one 8-bank pool,
                # so there are no psum pool boundaries anywhere.
                warm = mps.tile([128, 128], BF, tag="pg0", name="warm")
                for w in range(24):
                    nc.tensor.transpose(warm[:], ident, ident)
                auxpk = mps.tile([128, 512], F32, tag="pu0", name="auxpk")
                for hc in range(XC):
                    for t in range(TT):
                        for h in range(hc * HCB, (hc + 1) * HCB):
                            nc.tensor.matmul(
                                auxpk[:, t * 128:t * 128 + 88],
                                xth(h)[:, t * 128:(t + 1) * 128],
                                acall_h(h),
                                start=(t == 0 and h == 0),
                                stop=(t == TT - 1 and h == HB - 1))
                    # s0 gate mains ride along each xt chunk as it lands
                    for h in range(hc * HCB, (hc + 1) * HCB):
                        for i2 in range(NI2):
                            nc.tensor.matmul(psg0[i2],
                                             wt_g0[h // QW][:, h % QW,
                                                            i2 * 128:(i2 + 1) * 128],
                                             xth(h),
                                             start=(h == 0), stop=False)
                for t in range(TT):
                    emit_route(mps, "pu1", 2, auxtmp,
                               auxpk[:, t * 128:(t + 1) * 128],
                               80, 0, 8, t, ygT)
                proj_delta(psg0, bt_g0, ygT)
                st0 = emit_silu(0, psg0)
                for t in range(TT):
                    emit_route(mps, "pu1", 2, auxtmp,
                               auxpk[:, t * 128:(t + 1) * 128],
                               84, 40, 48, t, yuT)
                psu0 = [mps.tile([128, 512], F32, tag=f"pu{i2}",
                                 name=f"pu0_{i2}") for i2 in range(NI2)]
                proj_mains(psu0, wt_u0)
                proj_delta(psu0, bt_u0, yuT)
                emit_mult(0, st0, psu0)

                if True:
                    pre_bd0, pre_wd0 = None, []
                    for s in range(1, NS):
                        wt_g, bt_g = proj_weight_dmas(s, "g", wg_d, bg_d)
                        psg = [mps.tile([128, 512], F32, tag=f"pg{i2}",
                                        name=f"pg{s}_{i2}") for i2 in range(NI2)]
                        proj_mains(psg, wt_g)
                        proj_delta(psg, bt_g, ygT)
                        sts = emit_silu(s, psg)
                        wt_u, bt_u = proj_weight_dmas(s, "u", wu_d, bu_d)
                        psu = [mps.tile([128, 512], F32, tag=f"pu{i2}",
                                        name=f"pu{s}_{i2}") for i2 in range(NI2)]
                        proj_mains(psu, wt_u)
                        proj_delta(psu, bt_u, yuT)
                        emit_mult(s, sts, psu)
                        if s == 20:
                            # prefetch the first phase-2 tiles so chunk 0
                            # starts without waiting on the Sync queue.
                            a0, b0 = CH_H[0]
                            pre_bd0 = bstr2.tile([128, 512], BF, tag="bd2",
                                                 name="bd0")
                            nc.sync.dma_start(pre_bd0[:, :b0 - a0],
                                              bdcat_d[:, a0:b0])
                            wc0 = CH_F[0]
                            for ip in range(4):
                                wdt = wstr2.tile([128, 2 * 512], BF, tag="wd2",
                                                 name=f"wd0_{ip}")
                                nc.sync.dma_start(
                                    wdt[:, :2 * wc0],
                                    wdcat_d[:, wd_off[0] + ip * 2 * wc0:
                                            wd_off[0] + (ip + 1) * 2 * wc0])
                                pre_wd0.append(wdt)

                # ---- phase 2: down projection in N_CH column chunks,
                # sharing the SAME psum tag rings as phase 1 (no pool
                # boundary: chunk 0's psum slots were released back at
                # s=41, so phase 2 starts without draining phase 1).
                PT = ["pg0", "pg1", "pu0", "pu1"]
                out_v = out_d.rearrange("(t p) h -> p t h", p=128)

                def finish(c, pso, bdt):
                    a, b = CH_H[c]
                    wh = b - a
                    if c == N_CH - 1:
                        # combined store: one osb tile, copies alternating
                        # DVE/ACT, a single output descriptor.
                        osb = ost.tile([128, TT, 160], F32, tag="osl",
                                       name=f"osl{c}", bufs=1)
                        for t in range(TT):
                            nc.tensor.matmul(pso[t][:, :wh],
                                             ydT[:, t * 128:(t + 1) * 128],
                                             bdt[:, :wh], start=False, stop=True)
                            if t % 2 == 0:
                                nc.vector.tensor_copy(osb[:, t, :wh],
                                                      pso[t][:, :wh])
                            else:
                                nc.scalar.activation(
                                    osb[:, t, :wh], pso[t][:, :wh],
                                    mybir.ActivationFunctionType.Copy)
                        nc.sync.dma_start(out_v[:, :, a:b], osb[:])
                        return
                    for t in range(TT):
                        nc.tensor.matmul(pso[t][:, :wh],
                                         ydT[:, t * 128:(t + 1) * 128],
                                         bdt[:, :wh], start=False, stop=True)
                        osb = ost.tile([128, 512], F32, tag="os", name=f"os{c}_{t}")
                        nc.vector.tensor_copy(osb[:, :wh], pso[t][:, :wh])
                        nc.sync.dma_start(
                            out_d[t * 128:(t + 1) * 128, a:b], osb[:, :wh])

                for c in range(N_CH):
                    a, b = CH_H[c]
                    wh, wc = b - a, CH_F[c]
                    gp = 2 if wc >= 396 else 4   # i-blocks per descriptor
                    if c == 0 and pre_bd0 is not None:
                        bdt = pre_bd0
                    else:
                        bdt = bstr2.tile([128, 512], BF, tag="bd2",
                                         name=f"bd{c}")
                        nc.sync.dma_start(bdt[:, :wh], bdcat_d[:, a:b])
                    pso = [mps.tile([128, 512], F32, tag=PT[t],
                                    name=f"po{c}_{t}") for t in range(TT)]
                    for ip in range((IB + gp - 1) // gp):
                        ni = min(gp, IB - ip * gp)
                        if c == 0 and ip < len(pre_wd0):
                            wdt = pre_wd0[ip]
                        else:
                            wdt = wstr2.tile([128, 2 * 512], BF, tag="wd2",
                                             name=f"wd{c}_{ip}")
                            nc.sync.dma_start(
                                wdt[:, :ni * wc],
                                wdcat_d[:, wd_off[c] + ip * gp * wc:
                                        wd_off[c] + (ip * gp + ni) * wc])
                        for j in range(ni):
                            i = gp * ip + j
                            for t in range(TT):
                                nc.tensor.matmul(
                                    pso[t][:, :wc],
                                    inter_sb[:, i, t * 128:(t + 1) * 128],
                                    wdt[:, j * wc:(j + 1) * wc],
                                    start=(i == 0), stop=False)
                    if c == 0:
                        # down-routing straight from the psum aux columns;
                        # the transposes borrow the next ring generation.
                        for t in range(TT):
                            emit_route(mps, PT[t], 2, auxtmp, pso[t],
                                       352, 356, 364, t, ydT)
                    finish(c, pso, bdt)
    nc.finalize()
    return nc


def get_nc(H, I):
    key = (H, I)
    if key not in _nc_cache:
        _nc_cache[key] = build_kernel(H, I)
    return _nc_cache[key]


def _prep_weights(Wg, Wu, Wd, va_gate_A, va_gate_B, va_up_A, va_up_B,
                  va_down_A, va_down_B, router_gate, tm_gate_A, tm_gate_B,
                  router_up, tm_up_A, tm_up_B, router_down, tm_down_A, tm_down_B):
    I, H = Wg.shape
    HB, IB = H // 128, I // 128
    NS = I // SW

    def tile_w_ih(W):  # [I,H] -> [NS,128,HB,SW]; w[s,p,h,c]=W[s*SW+c, h*128+p]
        return np.ascontiguousarray(
            W.reshape(NS, SW, HB, 128).transpose(0, 3, 2, 1)).astype(BF16)

    def tile_bcat(vB, tB, rows):  # -> [nblk,128,blk]; padded 2*[vB|tB_e].T
        out_dim = vB.shape[0]
        Bcat = np.concatenate([vB] + [tB[e] for e in range(E)], axis=1)  # [out,40]
        Bp = np.zeros((128, out_dim), np.float32)
        Bp[:40, :] = SCALE * Bcat.T
        blk = out_dim // rows
        return np.ascontiguousarray(
            Bp.reshape(128, rows, blk).transpose(1, 0, 2)).astype(BF16)

    A_all = np.concatenate([va_gate_A, tm_gate_A.reshape(E * R, H),
                            va_up_A, tm_up_A.reshape(E * R, H),
                            router_gate, router_up], axis=0)  # [88,H]
    acatall = np.ascontiguousarray(
        A_all.T.reshape(HB, 128, 88).transpose(1, 0, 2)).astype(BF16)
    A_d = np.concatenate([router_down, va_down_A,
                          tm_down_A.reshape(E * R, I)], axis=0)  # [44,I]

    # down weights in column chunks; chunk 0 carries the down-aux columns
    parts = []
    for c, (a, b) in enumerate(CH_H):
        cols = Wd[a:b, :]                                    # [wh, I]
        if c == 0:
            cols = np.concatenate([cols, A_d], axis=0)       # [wh+44, I]
        wc = cols.shape[0]
        t = cols.T.reshape(IB, 128, wc).transpose(1, 0, 2)   # [128,IB,wc]
        parts.append(t.reshape(128, IB * wc))
    wdcat = np.ascontiguousarray(np.concatenate(parts, axis=1)).astype(BF16)

    Bcat_d = np.concatenate([va_down_B] + [tm_down_B[e] for e in range(E)],
                            axis=1)                          # [H,40]
    bdcat = np.zeros((128, H), np.float32)
    bdcat[:40, :] = SCALE * Bcat_d.T
    bdcat = np.ascontiguousarray(bdcat).astype(BF16)

    return {
        "acatall": acatall,
        "wg": tile_w_ih(Wg),
        "wu": tile_w_ih(Wu),
        "bg": tile_bcat(va_gate_B, tm_gate_B, NS),
        "bu": tile_bcat(va_up_B, tm_up_B, NS),
        "wdcat": wdcat,
        "bdcat": bdcat,
    }


def _prep_core_inputs(x, image_mask, weights, n_cores):
    Bb, S, H = x.shape
    HB = H // 128
    xf = np.asarray(x, np.float32).reshape(-1, H)
    m = np.asarray(image_mask).reshape(-1).astype(np.float32)
    in_maps = []
    for c in range(n_cores):
        sh = xf[c * T:(c + 1) * T]                      # [T,H]
        xt = np.ascontiguousarray(
            sh.T.reshape(HB, 128, T).transpose(1, 0, 2)).astype(BF16)
        mc = m[c * T:(c + 1) * T].reshape(TT, 128).T    # [128,TT]
        maskf = np.ascontiguousarray(
            np.concatenate([mc, 1.0 - mc], axis=1)).astype(np.float32)
        in_maps.append({"xt": xt, "maskf": maskf, **weights})
    return in_maps


def run(x, image_mask, weights_raw, trace=False):
    Bb, S, H = x.shape
    I = weights_raw["Wg"].shape[0]
    nc = get_nc(H, I)
    weights = _prep_weights(**weights_raw)
    in_maps = _prep_core_inputs(x, image_mask, weights, NCORES)
    res = run_bass_kernel_spmd(nc, in_maps, list(range(NCORES)), trace=trace)
    out = np.concatenate([r["out"] for r in res.results], axis=0)
    return out.reshape(Bb, S, H).astype(np.float32), res


def kernel(x, image_mask, Wg, Wu, Wd,
           va_gate_A, va_gate_B, va_up_A, va_up_B, va_down_A, va_down_B,
           router_gate, tm_gate_A, tm_gate_B,
           router_up, tm_up_A, tm_up_B,
           router_down, tm_down_A, tm_down_B):
    weights_raw = dict(
        Wg=np.asarray(Wg, np.float32), Wu=np.asarray(Wu, np.float32),
        Wd=np.asarray(Wd, np.float32),
        va_gate_A=np.asarray(va_gate_A), va_gate_B=np.asarray(va_gate_B),
        va_up_A=np.asarray(va_up_A), va_up_B=np.asarray(va_up_B),
        va_down_A=np.asarray(va_down_A), va_down_B=np.asarray(va_down_B),
        router_gate=np.asarray(router_gate), tm_gate_A=np.asarray(tm_gate_A),
        tm_gate_B=np.asarray(tm_gate_B),
        router_up=np.asarray(router_up), tm_up_A=np.asarray(tm_up_A),
        tm_up_B=np.asarray(tm_up_B),
        router_down=np.asarray(router_down), tm_down_A=np.asarray(tm_down_A),
        tm_down_B=np.asarray(tm_down_B),
    )
    out, _ = run(np.asarray(x), np.asarray(image_mask), weights_raw, trace=False)
    return out


# revision 24
# speedup vs baseline: 1.1122x; 1.0003x over previous
"""Trainium2 Bass kernel for nn_MoDEChameleonMLP (MoDE Chameleon MLP).

Math (per token n):
  gate = x@Wg.T + delta_g(x); up = x@Wu.T + delta_u(x)
  inter = silu(gate)*up
  out  = inter@Wd.T + delta_d(inter)
where delta(v) = mask ? 2*(v@vA.T)@vB.T : 2*sum_e softmax(v@router.T)_e (v@A_e.T)@B_e.T

Implementation: token(B*S)-sharding across 8 cores (512 tokens/core, no
collectives). Each core:
  aux:    t = x@Acat.T (rank-40 LoRA bases + router logits for gate/up),
          h-chunked over the streaming x DMA, all four token-tiles packed
          into a single PSUM bank (one start zeroes the bank; disjoint
          column ranges accumulate independently). Routing (softmax +
          mask combine) on DVE, transposed to [40,T] via PE+identity.
  phase1: gate/up = W-stationary matmuls producing [I-part, token] tiles,
          plus one extra K=128 matmul with the (pre-scaled) LoRA B matrix
          and y -> the full delta. s=0's weight DMAs and gate psum tags
          are pre-reserved before the aux scope so neither SBUF nor PSUM
          address reuse serializes phase1 behind the routing chains.
  phase2: down projection in 9 column-chunks; the first chunk's weight
          tile carries 44 extra columns holding the down-routing A
          matrices, so the whole down aux projection rides the existing
          matmuls (+1% rows) instead of a separate 344-matmul pass.
          Down-routing reads those PSUM columns directly; its transposes
          borrow the second generation of the output psum rings.
All matmuls bf16 with fp32 PSUM accumulation. Weights are host-side
transposed/pre-tiled so every device DMA is wide contiguous lines.
"""
import os
import sys

for p in ("/root/.axon_site/_ro/trn_rl_repo", "/opt/trn_rl_repo"):
    if os.path.isdir(p) and p not in sys.path:
        sys.path.append(p)

import numpy as np
import ml_dtypes

import concourse.bass as bass  # noqa: E402
import concourse.tile as tile  # noqa: E402
from concourse import bacc, mybir  # noqa: E402
from concourse.bass_utils import run_bass_kernel_spmd  # noqa: E402
from concourse.masks import make_identity  # noqa: E402

BF16 = ml_dtypes.bfloat16
BF = mybir.dt.bfloat16
F32 = mybir.dt.float32

NCORES = 8
T = 512          # tokens per core
TT = T // 128
SW = 256         # i-super width (2 x 128 psum tiles), divides 11008
E, R = 4, 8
SCALE = 2.0

# phase2 column chunking: chunk 0 carries 352 h-cols + 44 down-aux cols,
# then seven 512-wide chunks and a narrow 160-wide tail chunk (so the
# final psum drain after the last matmul is short). 352+7*512+160 = 4096.
CH_H = ([(0, 352)] + [(352 + 512 * k, 352 + 512 * (k + 1)) for k in range(7)]
        + [(3936, 4096)])
CH_F = [396] + [512] * 7 + [160]  # psum free width per chunk
N_CH = len(CH_H)

_nc_cache = {}


def build_kernel(H, I):
    HB, IB = H // 128, I // 128
    NS = I // SW
    NI2 = SW // 128
    XC = 4                        # xt dma chunks
    HCB = HB // XC                # h-blocks per xt chunk
    QW = 8                        # h-blocks per weight dma descriptor
    NQ = HB // QW
    wd_off = [86 * sum(CH_F[:c]) for c in range(N_CH)]
    WD_TOT = 86 * sum(CH_F)

    nc = bacc.Bacc(None, target_bir_lowering=False)
    xt_d = nc.declare_dram_parameter("xt", [128, HB, T], BF, isOutput=False)
    mask_d = nc.declare_dram_parameter("maskf", [128, 2 * TT], F32, isOutput=False)
    acall_d = nc.declare_dram_parameter("acatall", [128, HB, 88], BF, isOutput=False)
    wg_d = nc.declare_dram_parameter("wg", [NS, 128, HB, SW], BF, isOutput=False)
    wu_d = nc.declare_dram_parameter("wu", [NS, 128, HB, SW], BF, isOutput=False)
    bg_d = nc.declare_dram_parameter("bg", [NS, 128, SW], BF, isOutput=False)
    bu_d = nc.declare_dram_parameter("bu", [NS, 128, SW], BF, isOutput=False)
    wdcat_d = nc.declare_dram_parameter("wdcat", [128, WD_TOT], BF, isOutput=False)
    bdcat_d = nc.declare_dram_parameter("bdcat", [128, H], BF, isOutput=False)
    out_d = nc.declare_dram_parameter("out", [T, H], F32, isOutput=True)

    with tile.TileContext(nc) as tc:
        # wstr/wstr2/bstr2 are opened up-front so their SBUF addresses are
        # disjoint from every scoped pool: their DMAs then have no
        # address-release deps and prefetch freely across phase boundaries.
        with tc.tile_pool(name="const", bufs=1) as constp, \
             tc.tile_pool(name="wstr", bufs=10) as wstr, \
             tc.tile_pool(name="wstr2", bufs=10) as wstr2, \
             tc.tile_pool(name="bstr2", bufs=2) as bstr2:
            # ---- input DMAs, issue order matters: acall + xt chunks first
            # so the aux pass starts ASAP; s=0 weights follow immediately.
            acallc, xtc, wt_g0 = [], [], []

            def wg0_tile(q):
                wq = wstr.tile([128, QW, SW], BF, tag="wt", name=f"wg0_{q}")
                nc.sync.dma_start(wq[:], wg_d[0, :, q * QW:(q + 1) * QW, :])
                wt_g0.append(wq)

            for c in range(XC):
                ac = constp.tile([128, HCB, 88], BF, tag=f"acc{c}",
                                 name=f"acallc{c}")
                nc.sync.dma_start(ac[:], acall_d[:, c * HCB:(c + 1) * HCB, :])
                acallc.append(ac)
                xs = constp.tile([128, HCB, T], BF, tag=f"xtc{c}", name=f"xtc{c}")
                nc.sync.dma_start(xs[:], xt_d[:, c * HCB:(c + 1) * HCB, :])
                xtc.append(xs)
                wg0_tile(c)
            mask_sb = constp.tile([128, 2 * TT], F32)
            nc.sync.dma_start(mask_sb[:], mask_d[:])

            def xth(h):
                return xtc[h // HCB][:, h % HCB, :]

            def acall_h(h):
                return acallc[h // HCB][:, h % HCB, :]

            ident = constp.tile([128, 128], BF)
            make_identity(nc, ident)
            ygT = constp.tile([128, T], BF)
            warm_pending = True
            yuT = constp.tile([128, T], BF)
            ydT = constp.tile([128, T], BF)
            for y in (ygT, yuT, ydT):
                nc.vector.memset(y[:], 0.0)
            inter_sb = constp.tile([128, IB, T], BF)

            def emit_route(tpp, tptag, tpbufs, auxtmp, ps, lo, vo, eo, t, yT):
                """softmax(ps[:,lo:lo+4]) routing + mask combine -> y, then
                transpose y[128,40] into yT[0:40, t*128:(t+1)*128] via PE."""
                rmaxn = auxtmp.tile([128, 1], F32, tag="rmaxn", name=f"rx{t}")
                nc.vector.tensor_reduce(rmaxn, ps[:, lo:lo + 4],
                                        axis=mybir.AxisListType.X,
                                        op=mybir.AluOpType.max, negate=True)
                ee = auxtmp.tile([128, 4], F32, tag="ee", name=f"ee{t}")
                se = auxtmp.tile([128, 1], F32, tag="se", name=f"se{t}")
                nc.scalar.activation(ee, ps[:, lo:lo + 4],
                                     mybir.ActivationFunctionType.Exp,
                                     bias=rmaxn, accum_out=se)
                rec = auxtmp.tile([128, 1], F32, tag="rec", name=f"rc{t}")
                nc.vector.reciprocal(rec, se)
                r1m = auxtmp.tile([128, 1], F32, tag="r1m", name=f"rm{t}")
                nc.vector.tensor_tensor(r1m, rec, mask_sb[:, TT + t:TT + t + 1],
                                        mybir.AluOpType.mult)
                we = auxtmp.tile([128, 4], F32, tag="we", name=f"we{t}")
                nc.vector.tensor_scalar(we, ee, r1m, None, mybir.AluOpType.mult)
                yt = auxtmp.tile([128, 40], BF, tag="yt", name=f"yt{t}")
                nc.vector.tensor_scalar(yt[:, 0:8], ps[:, vo:vo + 8],
                                        mask_sb[:, t:t + 1], None,
                                        mybir.AluOpType.mult)
                for j in range(E):
                    nc.vector.tensor_scalar(yt[:, 8 + 8 * j:16 + 8 * j],
                                            ps[:, eo + 8 * j:eo + 8 * (j + 1)],
                                            we[:, j:j + 1], None,
                                            mybir.AluOpType.mult)
                tp = tpp.tile([128, 128], BF, tag=tptag, name=f"tp{t}",
                              bufs=tpbufs)
                nc.tensor.transpose(tp[:40, :], yt[:], ident)
                nc.vector.tensor_copy(yT[0:40, t * 128:(t + 1) * 128], tp[:40, :])

            with tc.tile_pool(name="bstr", bufs=3) as bstr, \
                 tc.tile_pool(name="etmp", bufs=4) as etmp, \
                 tc.tile_pool(name="ost", bufs=2) as ost, \
                 tc.tile_pool(name="auxtmp", bufs=2) as auxtmp, \
                 tc.tile_pool(name="mps", bufs=2, space="PSUM") as mps:

                def proj_weight_dmas(s, proj, w_dram, b_dram):
                    wt = []
                    for q in range(NQ):
                        wq = wstr.tile([128, QW, SW], BF, tag="wt",
                                       name=f"w{proj}{s}_{q}")
                        nc.sync.dma_start(wq[:], w_dram[s, :, q * QW:(q + 1) * QW, :])
                        wt.append(wq)
                    bt = bstr.tile([128, SW], BF, tag="bt", name=f"b{proj}{s}")
                    nc.sync.dma_start(bt[:], b_dram[s])
                    return wt, bt

                def proj_mains(pss, wt):
                    for h in range(HB):
                        for i2 in range(NI2):
                            nc.tensor.matmul(pss[i2],
                                             wt[h // QW][:, h % QW,
                                                         i2 * 128:(i2 + 1) * 128],
                                             xth(h),
                                             start=(h == 0), stop=False)

                def proj_delta(pss, bt, yT):
                    for i2 in range(NI2):
                        nc.tensor.matmul(pss[i2], bt[:, i2 * 128:(i2 + 1) * 128],
                                         yT[:], start=False, stop=True)

                # wg0 was issued interleaved with the xt chunks above; the
                # rest of s=0's weights stream right behind it.
                bt_g0 = bstr.tile([128, SW], BF, tag="bt", name="bg0")
                nc.sync.dma_start(bt_g0[:], bg_d[0])
                wt_u0, bt_u0 = proj_weight_dmas(0, "u", wu_d, bu_d)

                # ---- aux scope: the packed aux bank + route-transpose
                # staging (3 psum banks). s0's gate psums claim the pg tags
                # of the main ring first; the pu tags are claimed after this
                # scope closes and reuse its banks (Tile inserts the
                # address anti-deps, which are long satisfied by then).
                psg0 = [mps.tile([128, 512], F32, tag=f"pg{i2}",
                                 name=f"pg0_{i2}") for i2 in range(NI2)]

                def emit_silu(s, psg):
                    sts = []
                    for i2 in range(NI2):
                        st = etmp.tile([128, T], F32, tag="silu",
                                       name=f"si{s}_{i2}")
                        nc.scalar.activation(st[:], psg[i2][:, :T],
                                             mybir.ActivationFunctionType.Silu)
                        sts.append(st)
                    return sts

                def emit_mult(s, sts, psu):
                    for i2 in range(NI2):
                        i = s * NI2 + i2
                        nc.vector.tensor_tensor(inter_sb[:, i, :], sts[i2][:],
                                                psu[i2][:, :T],
                                                mybir.AluOpType.mult)

                # aux bank = pu0's first generation; route-transpose staging
                # rides pu1's ring. Everything lives in the one 8-bank pool,
                # so there are no psum pool boundaries anywhere.
                warm = mps.tile([128, 128], BF, tag="pg0", name="warm")
                for w in range(24):
                    nc.tensor.transpose(warm[:], ident, ident)
                auxpk = mps.tile([128, 512], F32, tag="pu0", name="auxpk")
                for hc in range(XC):
                    for t in range(TT):
                        for h in range(hc * HCB, (hc + 1) * HCB):
                            nc.tensor.matmul(
                                auxpk[:, t * 128:t * 128 + 88],
                                xth(h)[:, t * 128:(t + 1) * 128],
                                acall_h(h),
                                start=(t == 0 and h == 0),
                                stop=(t == TT - 1 and h == HB - 1))
                    # s0 gate mains ride along each xt chunk as it lands
                    for h in range(hc * HCB, (hc + 1) * HCB):
                        for i2 in range(NI2):
                            nc.tensor.matmul(psg0[i2],
                                             wt_g0[h // QW][:, h % QW,
                                                            i2 * 128:(i2 + 1) * 128],
                                             xth(h),
                                             start=(h == 0), stop=False)
                for t in range(TT):
                    emit_route(mps, "pu1", 2, auxtmp,
                               auxpk[:, t * 128:(t + 1) * 128],
                               80, 0, 8, t, ygT)
                proj_delta(psg0, bt_g0, ygT)
                st0 = emit_silu(0, psg0)
                for t in range(TT):
                    emit_route(mps, "pu1", 2, auxtmp,
                               auxpk[:, t * 128:(t + 1) * 128],
                               84, 40, 48, t, yuT)
                psu0 = [mps.tile([128, 512], F32, tag=f"pu{i2}",
                                 name=f"pu0_{i2}") for i2 in range(NI2)]
                proj_mains(psu0, wt_u0)
                proj_delta(psu0, bt_u0, yuT)
                emit_mult(0, st0, psu0)

                if True:
                    pre_bd0, pre_wd0 = None, []
                    for s in range(1, NS):
                        wt_g, bt_g = proj_weight_dmas(s, "g", wg_d, bg_d)
                        psg = [mps.tile([128, 512], F32, tag=f"pg{i2}",
                                        name=f"pg{s}_{i2}") for i2 in range(NI2)]
                        proj_mains(psg, wt_g)
                        proj_delta(psg, bt_g, ygT)
                        sts = emit_silu(s, psg)
                        wt_u, bt_u = proj_weight_dmas(s, "u", wu_d, bu_d)
                        psu = [mps.tile([128, 512], F32, tag=f"pu{i2}",
                                        name=f"pu{s}_{i2}") for i2 in range(NI2)]
                        proj_mains(psu, wt_u)
                        proj_delta(psu, bt_u, yuT)
                        emit_mult(s, sts, psu)
                        if s == 20:
                            # prefetch the first phase-2 tiles so chunk 0
                            # starts without waiting on the Sync queue.
                            a0, b0 = CH_H[0]
                            pre_bd0 = bstr2.tile([128, 512], BF, tag="bd2",
                                                 name="bd0")
                            nc.sync.dma_start(pre_bd0[:, :b0 - a0],
                                              bdcat_d[:, a0:b0])
                            wc0 = CH_F[0]
                            for ip in range(4):
                                wdt = wstr2.tile([128, 2 * 512], BF, tag="wd2",
                                                 name=f"wd0_{ip}")
                                nc.sync.dma_start(
                                    wdt[:, :2 * wc0],
                                    wdcat_d[:, wd_off[0] + ip * 2 * wc0:
                                            wd_off[0] + (ip + 1) * 2 * wc0])
                                pre_wd0.append(wdt)

                # ---- phase 2: down projection in N_CH column chunks,
                # sharing the SAME psum tag rings as phase 1 (no pool
                # boundary: chunk 0's psum slots were released back at
                # s=41, so phase 2 starts without draining phase 1).
                PT = ["pg0", "pg1", "pu0", "pu1"]
                out_v = out_d.rearrange("(t p) h -> p t h", p=128)

                def finish(c, pso, bdt):
                    a, b = CH_H[c]
                    wh = b - a
                    if c == N_CH - 1:
                        # combined store: one osb tile, copies alternating
                        # DVE/ACT, a single output descriptor.
                        osb = ost.tile([128, TT, 160], F32, tag="osl",
                                       name=f"osl{c}", bufs=1)
                        for t in range(TT):
                            nc.tensor.matmul(pso[t][:, :wh],
                                             ydT[:, t * 128:(t + 1) * 128],
                                             bdt[:, :wh], start=False, stop=True)
                            if t % 2 == 0:
                                nc.vector.tensor_copy(osb[:, t, :wh],
                                                      pso[t][:, :wh])
                            else:
                                nc.scalar.activation(
                                    osb[:, t, :wh], pso[t][:, :wh],
                                    mybir.ActivationFunctionType.Copy)
                        nc.sync.dma_start(out_v[:, :, a:b], osb[:])
                        return
                    for t in range(TT):
                        nc.tensor.matmul(pso[t][:, :wh],
                                         ydT[:, t * 128:(t + 1) * 128],
                                         bdt[:, :wh], start=False, stop=True)
                        osb = ost.tile([128, 512], F32, tag="os", name=f"os{c}_{t}")
                        nc.vector.tensor_copy(osb[:, :wh], pso[t][:, :wh])
                        nc.sync.dma_start(
                            out_d[t * 128:(t + 1) * 128, a:b], osb[:, :wh])

                for c in range(N_CH):
                    a, b = CH_H[c]
                    wh, wc = b - a, CH_F[c]
                    gp = 2 if wc >= 396 else 4   # i-blocks per descriptor
                    if c == 0 and pre_bd0 is not None:
                        bdt = pre_bd0
                    else:
                        bdt = bstr2.tile([128, 512], BF, tag="bd2",
                                         name=f"bd{c}")
                        nc.sync.dma_start(bdt[:, :wh], bdcat_d[:, a:b])
                    pso = [mps.tile([128, 512], F32, tag=PT[t],
                                    name=f"po{c}_{t}") for t in range(TT)]
                    for ip in range((IB + gp - 1) // gp):
                        ni = min(gp, IB - ip * gp)
                        if c == 0 and ip < len(pre_wd0):
                            wdt = pre_wd0[ip]
                        else:
                            wdt = wstr2.tile([128, 2 * 512], BF, tag="wd2",
                                             name=f"wd{c}_{ip}")
                            nc.sync.dma_start(
                                wdt[:, :ni * wc],
                                wdcat_d[:, wd_off[c] + ip * gp * wc:
                                        wd_off[c] + (ip * gp + ni) * wc])
                        for j in range(ni):
                            i = gp * ip + j
                            for t in range(TT):
                                nc.tensor.matmul(
                                    pso[t][:, :wc],
                                    inter_sb[:, i, t * 128:(t + 1) * 128],
                                    wdt[:, j * wc:(j + 1) * wc],
                                    start=(i == 0), stop=False)
                    if c == 0:
                        # down-routing straight from the psum aux columns;
                        # the transposes borrow the next ring generation.
                        for t in range(TT):
                            emit_route(mps, PT[t], 2, auxtmp, pso[t],
                                       352, 356, 364, t, ydT)
                    finish(c, pso, bdt)
    nc.finalize()
    return nc


def get_nc(H, I):
    key = (H, I)
    if key not in _nc_cache:
        _nc_cache[key] = build_kernel(H, I)
    return _nc_cache[key]


def _prep_weights(Wg, Wu, Wd, va_gate_A, va_gate_B, va_up_A, va_up_B,
                  va_down_A, va_down_B, router_gate, tm_gate_A, tm_gate_B,
                  router_up, tm_up_A, tm_up_B, router_down, tm_down_A, tm_down_B):
    I, H = Wg.shape
    HB, IB = H // 128, I // 128
    NS = I // SW

    def tile_w_ih(W):  # [I,H] -> [NS,128,HB,SW]; w[s,p,h,c]=W[s*SW+c, h*128+p]
        return np.ascontiguousarray(
            W.reshape(NS, SW, HB, 128).transpose(0, 3, 2, 1)).astype(BF16)

    def tile_bcat(vB, tB, rows):  # -> [nblk,128,blk]; padded 2*[vB|tB_e].T
        out_dim = vB.shape[0]
        Bcat = np.concatenate([vB] + [tB[e] for e in range(E)], axis=1)  # [out,40]
        Bp = np.zeros((128, out_dim), np.float32)
        Bp[:40, :] = SCALE * Bcat.T
        blk = out_dim // rows
        return np.ascontiguousarray(
            Bp.reshape(128, rows, blk).transpose(1, 0, 2)).astype(BF16)

    A_all = np.concatenate([va_gate_A, tm_gate_A.reshape(E * R, H),
                            va_up_A, tm_up_A.reshape(E * R, H),
                            router_gate, router_up], axis=0)  # [88,H]
    acatall = np.ascontiguousarray(
        A_all.T.reshape(HB, 128, 88).transpose(1, 0, 2)).astype(BF16)
    A_d = np.concatenate([router_down, va_down_A,
                          tm_down_A.reshape(E * R, I)], axis=0)  # [44,I]

    # down weights in column chunks; chunk 0 carries the down-aux columns
    parts = []
    for c, (a, b) in enumerate(CH_H):
        cols = Wd[a:b, :]                                    # [wh, I]
        if c == 0:
            cols = np.concatenate([cols, A_d], axis=0)       # [wh+44, I]
        wc = cols.shape[0]
        t = cols.T.reshape(IB, 128, wc).transpose(1, 0, 2)   # [128,IB,wc]
        parts.append(t.reshape(128, IB * wc))
    wdcat = np.ascontiguousarray(np.concatenate(parts, axis=1)).astype(BF16)

    Bcat_d = np.concatenate([va_down_B] + [tm_down_B[e] for e in range(E)],
                            axis=1)                          # [H,40]
    bdcat = np.zeros((128, H), np.float32)
    bdcat[:40, :] = SCALE * Bcat_d.T
    bdcat = np.ascontiguousarray(bdcat).astype(BF16)

    return {
        "acatall": acatall,
        "wg": tile_w_ih(Wg),
        "wu": tile_w_ih(Wu),
        "bg": tile_bcat(va_gate_B, tm_gate_B, NS),
        "bu": tile_bcat(va_up_B, tm_up_B, NS),
        "wdcat": wdcat,
        "bdcat": bdcat,
    }


def _prep_core_inputs(x, image_mask, weights, n_cores):
    Bb, S, H = x.shape
    HB = H // 128
    xf = np.asarray(x, np.float32).reshape(-1, H)
    m = np.asarray(image_mask).reshape(-1).astype(np.float32)
    in_maps = []
    for c in range(n_cores):
        sh = xf[c * T:(c + 1) * T]                      # [T,H]
        xt = np.ascontiguousarray(
            sh.T.reshape(HB, 128, T).transpose(1, 0, 2)).astype(BF16)
        mc = m[c * T:(c + 1) * T].reshape(TT, 128).T    # [128,TT]
        maskf = np.ascontiguousarray(
            np.concatenate([mc, 1.0 - mc], axis=1)).astype(np.float32)
        in_maps.append({"xt": xt, "maskf": maskf, **weights})
    return in_maps


def run(x, image_mask, weights_raw, trace=False):
    Bb, S, H = x.shape
    I = weights_raw["Wg"].shape[0]
    nc = get_nc(H, I)
    weights = _prep_weights(**weights_raw)
    in_maps = _prep_core_inputs(x, image_mask, weights, NCORES)
    res = run_bass_kernel_spmd(nc, in_maps, list(range(NCORES)), trace=trace)
    out = np.concatenate([r["out"] for r in res.results], axis=0)
    return out.reshape(Bb, S, H).astype(np.float32), res


def kernel(x, image_mask, Wg, Wu, Wd,
           va_gate_A, va_gate_B, va_up_A, va_up_B, va_down_A, va_down_B,
           router_gate, tm_gate_A, tm_gate_B,
           router_up, tm_up_A, tm_up_B,
           router_down, tm_down_A, tm_down_B):
    weights_raw = dict(
        Wg=np.asarray(Wg, np.float32), Wu=np.asarray(Wu, np.float32),
        Wd=np.asarray(Wd, np.float32),
        va_gate_A=np.asarray(va_gate_A), va_gate_B=np.asarray(va_gate_B),
        va_up_A=np.asarray(va_up_A), va_up_B=np.asarray(va_up_B),
        va_down_A=np.asarray(va_down_A), va_down_B=np.asarray(va_down_B),
        router_gate=np.asarray(router_gate), tm_gate_A=np.asarray(tm_gate_A),
        tm_gate_B=np.asarray(tm_gate_B),
        router_up=np.asarray(router_up), tm_up_A=np.asarray(tm_up_A),
        tm_up_B=np.asarray(tm_up_B),
        router_down=np.asarray(router_down), tm_down_A=np.asarray(tm_down_A),
        tm_down_B=np.asarray(tm_down_B),
    )
    out, _ = run(np.asarray(x), np.asarray(image_mask), weights_raw, trace=False)
    return out
